# revision 7
# baseline (speedup 1.0000x reference)
"""Trainium2 Bass kernel for nn_RT_12068858101896 (dense transformer:
encoder + 2 levels of [LocalRNN(GRU,K=7) -> causal MHA -> FFN] + sigmoid head).

Distribution: data-parallel over batch. B=8 batch elements, one per NeuronCore.
Each core runs the full network on its [S=1536, IN=32] slice.

Per-core layout strategy:
 - residual h kept as [S, D] fp32 (12 tiles of [128,128]); layernorm stats +
   residual adds live here (per-partition stats via bn_stats/bn_aggr).
 - LN affine (a, b) folded into the consumer matmul weights/biases, so the LN
   body only computes t = (x - mean)/(std + eps).
 - matmul activations kept transposed [D, S] in bf16 (PE fast-weight-load only
   engages for non-fp32 operands; fp32 matmuls pay a serialized per-matmul
   4-byte weight load). PSUM accumulation stays fp32.
 - GRU: per step, gi + gh accumulate in one PSUM group (the Wih matmul is
   recomputed on a column-shifted window each step; left pad columns hold
   -b/a so the zero-pad of the original input is reproduced exactly).
 - attention: scores computed transposed [sk, sq] so softmax normalization is
   a column sum obtained free from an appended ones-column in the PV matmul;
   softmax skips max-subtraction (scores bounded ~2); causal mask applied as
   a triangular elementwise multiply on exp'd diagonal tiles.
"""
import math
import numpy as np
from contextlib import ExitStack

import concourse.bass as bass
import concourse.bacc as bacc
import concourse.tile as tile
from concourse import mybir
from concourse.bass_utils import run_bass_kernel_spmd
from concourse.masks import make_identity, make_upper_triangular

F32 = mybir.dt.float32
BF16 = mybir.dt.bfloat16
AF = mybir.ActivationFunctionType
ALU = mybir.AluOpType

USE_BF16 = True
CDT = BF16 if USE_BF16 else F32   # compute dtype for matmul operands

B, S, IN, D, H, K, NLVL = 8, 1536, 32, 128, 4, 7, 2
DFF = 4 * D          # 512
DK = D // H          # 32
EPS = 1e-6
P = 128
ST = S // P          # 12 s-tiles
NB = S // 512        # 3 free-dim blocks
NCORES = 8
PAD = K - 1          # 6

_CACHE = {}


def _ln_stats(nc, pool, h_sb):
    """Per s-tile (mean, 1/(std+eps)) for layernorm over D (free dim)."""
    mvs = pool.tile([P, ST, 2], F32, tag="ln_mvs")
    for t in range(ST):
        stats = pool.tile([P, 6], F32, tag="ln_stats", bufs=2)
        nc.vector.bn_stats(out=stats, in_=h_sb[:, t * P:(t + 1) * P])
        nc.vector.bn_aggr(out=mvs[:, t, :], in_=stats)
    inv = pool.tile([P, ST], F32, tag="ln_inv")
    # std = sqrt(var * D/(D-1)); inv = 1/(std + eps)
    nc.scalar.activation(inv, mvs[:, :, 1], AF.Sqrt, bias=0.0, scale=float(D) / (D - 1))
    nc.vector.tensor_scalar_add(inv, inv, EPS)
    nc.vector.reciprocal(inv, inv)
    return mvs, inv


def _ln_to_T(nc, pool, psum_pool, h_sb, ident_c, out_T, out_off):
    """Layernorm h_sb -> normalized t (CDT), transposed into out_T."""
    mvs, inv = _ln_stats(nc, pool, h_sb)
    for t in range(ST):
        tmp = pool.tile([P, P], CDT, tag="ln_tmp", bufs=2)
        nc.vector.tensor_scalar(
            out=tmp, in0=h_sb[:, t * P:(t + 1) * P],
            scalar1=mvs[:, t, 0:1], scalar2=inv[:, t:t + 1],
            op0=ALU.subtract, op1=ALU.mult,
        )
        tp = psum_pool.tile([P, P], CDT, tag="tp")
        nc.tensor.transpose(tp, tmp, ident_c)
        nc.vector.tensor_copy(out_T[:, out_off + t * P: out_off + (t + 1) * P], tp)


def _residual_add_T(nc, psum_pool, h_sb, src_T, ident_c):
    """h_sb[s, d] += src_T[d, s] via PE transposes (src in CDT)."""
    for t in range(ST):
        tp = psum_pool.tile([P, P], CDT, tag="tp")
        nc.tensor.transpose(tp, src_T[:, t * P:(t + 1) * P], ident_c)
        nc.vector.tensor_add(
            h_sb[:, t * P:(t + 1) * P], h_sb[:, t * P:(t + 1) * P], tp)


def _bias_col(nc, pool, dram_vec, tag):
    """Load a length-128 dram vector as a [128,1] fp32 per-partition column."""
    t = pool.tile([P, 1], F32, tag=tag)
    nc.sync.dma_start(out=t, in_=dram_vec)
    return t


def build():
    nc = bacc.Bacc()

    x_d = nc.declare_dram_parameter("x", [S, IN], F32, isOutput=False)
    encW_d = nc.declare_dram_parameter("enc_W", [IN, D], F32, isOutput=False)
    encb_d = nc.declare_dram_parameter("enc_b", [D], F32, isOutput=False)
    lra_d = nc.declare_dram_parameter("ln_rnn_a", [NLVL, D], F32, isOutput=False)
    lrb_d = nc.declare_dram_parameter("ln_rnn_b", [NLVL, D], F32, isOutput=False)
    wih_d = nc.declare_dram_parameter("gru_Wih", [NLVL, 3 * D, D], F32, isOutput=False)
    whh_d = nc.declare_dram_parameter("gru_Whh", [NLVL, 3 * D, D], F32, isOutput=False)
    bih_d = nc.declare_dram_parameter("gru_bih", [NLVL, 3 * D], F32, isOutput=False)
    bhh_d = nc.declare_dram_parameter("gru_bhh", [NLVL, 3 * D], F32, isOutput=False)
    laa_d = nc.declare_dram_parameter("ln_attn_a", [NLVL, D], F32, isOutput=False)
    lab_d = nc.declare_dram_parameter("ln_attn_b", [NLVL, D], F32, isOutput=False)
    wq_d = nc.declare_dram_parameter("Wq", [NLVL, D, D], F32, isOutput=False)
    bq_d = nc.declare_dram_parameter("bq", [NLVL, D], F32, isOutput=False)
    wk_d = nc.declare_dram_parameter("Wk", [NLVL, D, D], F32, isOutput=False)
    bk_d = nc.declare_dram_parameter("bk", [NLVL, D], F32, isOutput=False)
    wv_d = nc.declare_dram_parameter("Wv", [NLVL, D, D], F32, isOutput=False)
    bv_d = nc.declare_dram_parameter("bv", [NLVL, D], F32, isOutput=False)
    wo_d = nc.declare_dram_parameter("Wo", [NLVL, D, D], F32, isOutput=False)
    bo_d = nc.declare_dram_parameter("bo", [NLVL, D], F32, isOutput=False)
    lfa_d = nc.declare_dram_parameter("ln_ff_a", [NLVL, D], F32, isOutput=False)
    lfb_d = nc.declare_dram_parameter("ln_ff_b", [NLVL, D], F32, isOutput=False)
    w1_d = nc.declare_dram_parameter("ff_W1", [NLVL, D, DFF], F32, isOutput=False)
    b1_d = nc.declare_dram_parameter("ff_b1", [NLVL, DFF], F32, isOutput=False)
    w2_d = nc.declare_dram_parameter("ff_W2", [NLVL, DFF, D], F32, isOutput=False)
    b2_d = nc.declare_dram_parameter("ff_b2", [NLVL, D], F32, isOutput=False)
    outw_d = nc.declare_dram_parameter("out_W", [D, 1], F32, isOutput=False)
    outb_d = nc.declare_dram_parameter("out_b", [1], F32, isOutput=False)
    y_d = nc.declare_dram_parameter("y", [S, 1], F32, isOutput=True)

    with tile.TileContext(nc) as tc, ExitStack() as ctx:
        cpool = ctx.enter_context(tc.tile_pool(name="consts", bufs=1))
        spool = ctx.enter_context(tc.tile_pool(name="work", bufs=1))
        wpool = ctx.enter_context(tc.tile_pool(name="weights", bufs=2))

        ident = cpool.tile([P, P], F32)
        make_identity(nc, ident)
        if USE_BF16:
            ident_c = cpool.tile([P, P], CDT, name="ident_c")
            nc.vector.tensor_copy(ident_c, ident)
        else:
            ident_c = ident
        tri = cpool.tile([P, P], CDT)
        make_upper_triangular(nc, tri, val=1.0, diag=True)
        ones1h = cpool.tile([1, DK], F32)
        nc.vector.memset(ones1h, 1.0)

        h_sb = cpool.tile([P, S], F32)          # residual stream [s, d] tiles
        hT_sb = cpool.tile([P, S], F32)         # scratch transposed stream

        # ---- encoder: h = x @ enc_W + enc_b ----
        encW_f = spool.tile([IN, D], F32, tag="encW_f")
        nc.sync.dma_start(out=encW_f, in_=encW_d[:, :])
        encW = spool.tile([IN, D], CDT, tag="encW")
        nc.vector.tensor_copy(encW, encW_f)
        encb = _bias_col(nc, spool, encb_d[:], "encb")
        xT = spool.tile([IN, S], CDT, tag="xT")
        with tc.tile_pool(name="enc_ps", bufs=2, space="PSUM") as eps_pool:
            for t in range(ST):
                x_t = spool.tile([P, IN], F32, tag="x_t", bufs=3)
                nc.sync.dma_start(out=x_t, in_=x_d[t * P:(t + 1) * P, :])
                x_c = spool.tile([P, IN], CDT, tag="x_c", bufs=3)
                nc.vector.tensor_copy(x_c, x_t)
                tp = eps_pool.tile([IN, P], CDT, tag="xtp")
                nc.tensor.transpose(tp, x_c, ident_c)
                nc.vector.tensor_copy(xT[:, t * P:(t + 1) * P], tp)
            for b in range(NB):
                h0 = eps_pool.tile([P, 512], F32, tag="h0")
                nc.tensor.matmul(h0, lhsT=encW, rhs=xT[:, 512 * b:512 * (b + 1)],
                                 start=True, stop=True)
                nc.scalar.activation(hT_sb[:, 512 * b:512 * (b + 1)], h0,
                                     AF.Identity, bias=encb[:, 0:1], scale=1.0)
            for t in range(ST):
                tp2 = eps_pool.tile([P, P], F32, tag="tp")
                nc.tensor.transpose(tp2, hT_sb[:, t * P:(t + 1) * P], ident)
                nc.vector.tensor_copy(h_sb[:, t * P:(t + 1) * P], tp2)

        for lvl in range(NLVL):
            # ================= LocalRNN (GRU over K=7 causal window) ========
            lra = _bias_col(nc, wpool, lra_d[lvl, :], "lra")
            lrb = _bias_col(nc, wpool, lrb_d[lvl, :], "lrb")
            lrb_c = wpool.tile([P, 1], CDT, tag="lrb_c")
            nc.vector.tensor_copy(lrb_c, lrb)
            wihT = wpool.tile([P, 3 * D], CDT, tag="wihT")
            whhT = wpool.tile([P, 3 * D], CDT, tag="whhT")
            bihp = []      # bih + b @ Wih.T  (per gate chunk, [128,1] fp32)
            bhhc = []      # bhh chunks
            with tc.tile_pool(name="gw_ps", bufs=2, space="PSUM") as gw_ps:
                for c in range(3):
                    wtmp = wpool.tile([P, P], F32, tag="w_load", bufs=3)
                    nc.sync.dma_start(out=wtmp, in_=wih_d[lvl, P * c:P * (c + 1), :])
                    tp = gw_ps.tile([P, P], F32, tag="wtp")
                    nc.tensor.transpose(tp, wtmp, ident)
                    nc.vector.tensor_copy(wihT[:, P * c:P * (c + 1)], tp)
                    wtmp2 = wpool.tile([P, P], F32, tag="w_load", bufs=3)
                    nc.sync.dma_start(out=wtmp2, in_=whh_d[lvl, P * c:P * (c + 1), :])
                    tp2 = gw_ps.tile([P, P], F32, tag="wtp")
                    nc.tensor.transpose(tp2, wtmp2, ident)
                    nc.vector.tensor_copy(whhT[:, P * c:P * (c + 1)], tp2)
                for c in range(3):
                    bw_ps = gw_ps.tile([P, 1], F32, tag="bw")
                    nc.tensor.matmul(bw_ps, lhsT=wihT[:, P * c:P * (c + 1)], rhs=lrb_c,
                                     start=True, stop=True)
                    bi = _bias_col(nc, wpool, bih_d[lvl, P * c:P * (c + 1)], f"bih{c}")
                    bip = wpool.tile([P, 1], F32, tag=f"bihp{c}")
                    nc.vector.tensor_add(bip, bi, bw_ps)
                    bihp.append(bip)
                    bhhc.append(_bias_col(nc, wpool, bhh_d[lvl, P * c:P * (c + 1)], f"bhh{c}"))
                # scale wihT rows by ln_rnn_a (after the bias matmuls read it)
                nc.vector.tensor_scalar_mul(wihT, wihT, lra)
                rzbias = []
                for c in range(2):
                    rb = wpool.tile([P, 1], F32, tag=f"rzb{c}")
                    nc.vector.tensor_add(rb, bihp[c], bhhc[c])
                    rzbias.append(rb)
                # pad value: -b/a (emulates zero-pad of the pre-LN-affine input)
                pb = wpool.tile([P, 1], F32, tag="pb")
                nc.vector.reciprocal(pb, lra)
                nc.vector.tensor_mul(pb, pb, lrb)
                nc.vector.tensor_scalar_mul(pb, pb, -1.0)

            lnp = spool.tile([P, PAD + S], CDT, tag="lnp")
            with tc.tile_pool(name="ln_ps", bufs=2, space="PSUM") as ln_ps:
                _ln_to_T(nc, spool, ln_ps, h_sb, ident_c, lnp, PAD)
            nc.vector.tensor_copy(lnp[:, 0:PAD], pb[:, 0:1].to_broadcast([P, PAD]))

            hprev = None
            with tc.tile_pool(name="gru_ps", bufs=2, space="PSUM") as gps:
                for k in range(K):
                    hnew = spool.tile([P, S], CDT, tag="hst", bufs=2)
                    for b in range(NB):
                        c0, c1 = 512 * b, 512 * (b + 1)
                        col = k + 512 * b
                        ps_r = gps.tile([P, 512], F32, tag="ps_r")
                        ps_z = gps.tile([P, 512], F32, tag="ps_z")
                        ps_ni = gps.tile([P, 512], F32, tag="ps_ni")
                        rhs_x = lnp[:, col:col + 512]
                        nc.tensor.matmul(ps_r, lhsT=wihT[:, 0:P], rhs=rhs_x,
                                         start=True, stop=(k == 0))
                        nc.tensor.matmul(ps_z, lhsT=wihT[:, P:2 * P], rhs=rhs_x,
                                         start=True, stop=(k == 0))
                        nc.tensor.matmul(ps_ni, lhsT=wihT[:, 2 * P:3 * P], rhs=rhs_x,
                                         start=True, stop=True)
                        if k > 0:
                            ps_nh = gps.tile([P, 512], F32, tag="ps_nh")
                            nc.tensor.matmul(ps_r, lhsT=whhT[:, 0:P],
                                             rhs=hprev[:, c0:c1], start=False, stop=True)
                            nc.tensor.matmul(ps_z, lhsT=whhT[:, P:2 * P],
                                             rhs=hprev[:, c0:c1], start=False, stop=True)
                            nc.tensor.matmul(ps_nh, lhsT=whhT[:, 2 * P:3 * P],
                                             rhs=hprev[:, c0:c1], start=True, stop=True)
                        r_sb = spool.tile([P, 512], CDT, tag="gru_r", bufs=2)
                        nc.scalar.activation(r_sb, ps_r, AF.Sigmoid,
                                             bias=rzbias[0][:, 0:1], scale=1.0)
                        z_sb = spool.tile([P, 512], CDT, tag="gru_z", bufs=2)
                        nc.scalar.activation(z_sb, ps_z, AF.Sigmoid,
                                             bias=rzbias[1][:, 0:1], scale=1.0)
                        t_sb = spool.tile([P, 512], CDT, tag="gru_t", bufs=2)
                        if k > 0:
                            # t = (hn + bhh_n) * r
                            nc.vector.scalar_tensor_tensor(
                                out=t_sb, in0=ps_nh, scalar=bhhc[2][:, 0:1], in1=r_sb,
                                op0=ALU.add, op1=ALU.mult)
                        else:
                            nc.vector.tensor_scalar_mul(t_sb, r_sb, bhhc[2][:, 0:1])
                        t2_sb = spool.tile([P, 512], F32, tag="gru_t2", bufs=2)
                        nc.vector.tensor_add(t2_sb, t_sb, ps_ni)
                        nn_sb = spool.tile([P, 512], CDT, tag="gru_nn", bufs=2)
                        nc.scalar.activation(nn_sb, t2_sb, AF.Tanh,
                                             bias=bihp[2][:, 0:1], scale=1.0)
                        if k > 0:
                            d_sb = spool.tile([P, 512], CDT, tag="gru_d", bufs=2)
                            nc.vector.tensor_sub(d_sb, hprev[:, c0:c1], nn_sb)
                            nc.vector.tensor_mul(d_sb, z_sb, d_sb)
                            nc.vector.tensor_add(hnew[:, c0:c1], nn_sb, d_sb)
                        else:
                            # h' = (1-z) * nn
                            nc.vector.tensor_scalar(
                                out=z_sb, in0=z_sb, scalar1=-1.0, scalar2=1.0,
                                op0=ALU.mult, op1=ALU.add)
                            nc.vector.tensor_mul(hnew[:, c0:c1], z_sb, nn_sb)
                    hprev = hnew
            with tc.tile_pool(name="res_ps", bufs=2, space="PSUM") as rps:
                _residual_add_T(nc, rps, h_sb, hprev, ident_c)

            # ======================= causal attention =======================
            laa = _bias_col(nc, wpool, laa_d[lvl, :], "laa")
            lab = _bias_col(nc, wpool, lab_d[lvl, :], "lab")
            wq_f = wpool.tile([P, D], F32, tag="wq_f")
            nc.sync.dma_start(out=wq_f, in_=wq_d[lvl, :, :])
            wk_f = wpool.tile([P, D], F32, tag="wk_f")
            nc.sync.dma_start(out=wk_f, in_=wk_d[lvl, :, :])
            wv_f = wpool.tile([P, D], F32, tag="wv_f")
            nc.sync.dma_start(out=wv_f, in_=wv_d[lvl, :, :])
            wo_f = wpool.tile([P, D], F32, tag="wo_f")
            nc.sync.dma_start(out=wo_f, in_=wo_d[lvl, :, :])
            wo_h = []
            for h in range(H):
                woh_f = wpool.tile([DK, D], F32, tag=f"wo{h}_f")
                nc.sync.dma_start(out=woh_f, in_=wo_d[lvl, DK * h:DK * (h + 1), :])
                woh = wpool.tile([DK, D], CDT, tag=f"wo{h}")
                nc.vector.tensor_copy(woh, woh_f)
                wo_h.append(woh)
            with tc.tile_pool(name="aw_ps", bufs=2, space="PSUM") as aw_ps:
                def proj_bias(wmat_f, bdram, tag):
                    ps = aw_ps.tile([P, 1], F32, tag="pb_ps")
                    nc.tensor.matmul(ps, lhsT=wmat_f, rhs=lab, start=True, stop=True)
                    bcol = _bias_col(nc, wpool, bdram, tag + "_l")
                    bout = wpool.tile([P, 1], F32, tag=tag)
                    nc.vector.tensor_add(bout, bcol, ps)
                    return bout
                bqp = proj_bias(wq_f, bq_d[lvl, :], "bqp")
                bkp = proj_bias(wk_f, bk_d[lvl, :], "bkp")
                bvp = proj_bias(wv_f, bv_d[lvl, :], "bvp")
                wq = wpool.tile([P, D], CDT, tag="wq")
                nc.vector.tensor_scalar_mul(wq, wq_f, laa)
                wk = wpool.tile([P, D], CDT, tag="wk")
                nc.vector.tensor_scalar_mul(wk, wk_f, laa)
                wv = wpool.tile([P, D], CDT, tag="wv")
                nc.vector.tensor_scalar_mul(wv, wv_f, laa)
                # bo_tot = bo + Wo.T @ bv'
                bo_ps = aw_ps.tile([P, 1], F32, tag="pb_ps")
                nc.tensor.matmul(bo_ps, lhsT=wo_f, rhs=bvp, start=True, stop=True)
                bo_l = _bias_col(nc, wpool, bo_d[lvl, :], "bo_l")
                bo_tot = wpool.tile([P, 1], F32, tag="bo_tot")
                nc.vector.tensor_add(bo_tot, bo_l, bo_ps)

            lnTa = spool.tile([P, S], CDT, tag="lnTa")
            with tc.tile_pool(name="lna_ps", bufs=2, space="PSUM") as lna_ps:
                _ln_to_T(nc, spool, lna_ps, h_sb, ident_c, lnTa, 0)

            qT = spool.tile([P, S], CDT, tag="qT")
            kT = spool.tile([P, S], CDT, tag="kT")
            v_sb = spool.tile([P, ST, H, DK + 1], CDT, tag="v_sb")
            nc.vector.memset(v_sb[:, :, :, DK:DK + 1], 1.0)
            attnT = spool.tile([P, S], CDT, tag="attnT")
            with tc.tile_pool(name="qkv_ps", bufs=2, space="PSUM") as qkv_ps:
                for b in range(NB):
                    c0, c1 = 512 * b, 512 * (b + 1)
                    pq = qkv_ps.tile([P, 512], F32, tag="pq")
                    nc.tensor.matmul(pq, lhsT=wq, rhs=lnTa[:, c0:c1], start=True, stop=True)
                    nc.scalar.activation(qT[:, c0:c1], pq, AF.Identity,
                                         bias=bqp[:, 0:1], scale=1.0)
                    pk = qkv_ps.tile([P, 512], F32, tag="pk")
                    nc.tensor.matmul(pk, lhsT=wk, rhs=lnTa[:, c0:c1], start=True, stop=True)
                    nc.scalar.activation(kT[:, c0:c1], pk, AF.Identity,
                                         bias=bkp[:, 0:1], scale=1.0)
                for t in range(ST):
                    pv = qkv_ps.tile([P, P], F32, tag="pv")
                    nc.tensor.matmul(pv, lhsT=lnTa[:, t * P:(t + 1) * P], rhs=wv,
                                     start=True, stop=True)
                    nc.vector.tensor_copy(
                        v_sb[:, t, :, 0:DK],
                        pv.rearrange("p (h e) -> p h e", h=H))

            # PE operands may only start at partitions {0,32,64}; head 3 sits
            # at 96, so give it base-0 copies of its q/k rows.
            q3 = spool.tile([DK, S], CDT, tag="q3")
            nc.vector.tensor_copy(q3, qT[3 * DK:4 * DK, :])
            k3 = spool.tile([DK, S], CDT, tag="k3")
            nc.vector.tensor_copy(k3, kT[3 * DK:4 * DK, :])

            inv_scale = 1.0 / math.sqrt(DK)
            with tc.tile_pool(name="att_ps", bufs=2, space="PSUM") as aps:
                for b in range(NB):
                    c0, c1 = 512 * b, 512 * (b + 1)
                    attn_ps = aps.tile([P, 512], F32, tag="attn", bufs=2)
                    for h in range(H):
                        hp0, hp1 = DK * h, DK * (h + 1)
                        qh = q3 if h == 3 else qT[hp0:hp1, :]
                        kh = k3 if h == 3 else kT[hp0:hp1, :]
                        o_ps = aps.tile([DK + 1, 512], F32, tag="opv", bufs=2)
                        n_sk = 4 * b + 4
                        for t in range(n_sk):
                            nst = P * max(0, t - 4 * b)
                            s_ps = aps.tile([P, 512], F32, tag="qk", bufs=2)
                            nc.tensor.matmul(
                                s_ps[:, nst:512], lhsT=kh[:, t * P:(t + 1) * P],
                                rhs=qh[:, c0 + nst:c1], start=True, stop=True)
                            est = spool.tile([P, 512], CDT, tag="est", bufs=4)
                            if nst > 0:
                                nc.vector.memset(est[:, 0:nst], 0.0)
                            nc.scalar.activation(est[:, nst:512], s_ps[:, nst:512],
                                                 AF.Exp, bias=0.0, scale=inv_scale)
                            if t >= 4 * b:
                                j = t - 4 * b
                                nc.vector.tensor_mul(
                                    est[:, j * P:(j + 1) * P],
                                    est[:, j * P:(j + 1) * P], tri)
                            nc.tensor.matmul(o_ps, lhsT=v_sb[:, t, h, :], rhs=est,
                                             start=(t == 0), stop=(t == n_sk - 1))
                        inv_row = spool.tile([1, 512], F32, tag="inv_row", bufs=2)
                        nc.vector.reciprocal(inv_row, o_ps[DK:DK + 1, :])
                        invb = aps.tile([DK, 512], F32, tag="invb", bufs=2)
                        nc.tensor.matmul(invb, lhsT=ones1h, rhs=inv_row,
                                         start=True, stop=True)
                        # DVE may read only one non-scalar PSUM input: stage
                        # the broadcast reciprocal through SBUF first.
                        invb_sb = spool.tile([DK, 512], F32, tag="invb_sb", bufs=2)
                        nc.vector.tensor_copy(invb_sb, invb)
                        o_sb = spool.tile([DK, 512], CDT, tag="o_sb", bufs=2)
                        nc.vector.tensor_mul(o_sb, o_ps[0:DK, :], invb_sb)
                        nc.tensor.matmul(attn_ps, lhsT=wo_h[h], rhs=o_sb,
                                         start=(h == 0), stop=(h == H - 1))
                    nc.scalar.activation(attnT[:, c0:c1], attn_ps, AF.Identity,
                                         bias=bo_tot[:, 0:1], scale=1.0)
            with tc.tile_pool(name="resa_ps", bufs=2, space="PSUM") as rps:
                _residual_add_T(nc, rps, h_sb, attnT, ident_c)

            # ============================ FFN ===============================
            lfa = _bias_col(nc, wpool, lfa_d[lvl, :], "lfa")
            lfb = _bias_col(nc, wpool, lfb_d[lvl, :], "lfb")
            w1_f = wpool.tile([P, DFF], F32, tag="w1_f")
            nc.sync.dma_start(out=w1_f, in_=w1_d[lvl, :, :])
            w2_f = wpool.tile([P, 4, P], F32, tag="w2_f")
            for c in range(4):
                nc.sync.dma_start(out=w2_f[:, c, :], in_=w2_d[lvl, P * c:P * (c + 1), :])
            w2 = wpool.tile([P, 4, P], CDT, tag="w2")
            nc.vector.tensor_copy(w2, w2_f)
            b2c = _bias_col(nc, wpool, b2_d[lvl, :], "b2c")
            b1p = []
            with tc.tile_pool(name="fw_ps", bufs=2, space="PSUM") as fw_ps:
                for c in range(4):
                    ps = fw_ps.tile([P, 1], F32, tag="b1_ps")
                    nc.tensor.matmul(ps, lhsT=w1_f[:, P * c:P * (c + 1)], rhs=lfb,
                                     start=True, stop=True)
                    bcol = _bias_col(nc, wpool, b1_d[lvl, P * c:P * (c + 1)], f"b1l{c}")
                    bp = wpool.tile([P, 1], F32, tag=f"b1p{c}")
                    nc.vector.tensor_add(bp, bcol, ps)
                    b1p.append(bp)
                w1 = wpool.tile([P, DFF], CDT, tag="w1")
                nc.vector.tensor_scalar_mul(w1, w1_f, lfa)

            lnTf = spool.tile([P, S], CDT, tag="lnTa")
            with tc.tile_pool(name="lnf_ps", bufs=2, space="PSUM") as lnf_ps:
                _ln_to_T(nc, spool, lnf_ps, h_sb, ident_c, lnTf, 0)

            yT = spool.tile([P, S], CDT, tag="attnT")
            with tc.tile_pool(name="ffn_ps", bufs=1, space="PSUM") as fps:
                for b in range(NB):
                    c0, c1 = 512 * b, 512 * (b + 1)
                    u_tiles = []
                    for c in range(4):
                        up = fps.tile([P, 512], F32, tag=f"u{c}")
                        nc.tensor.matmul(up, lhsT=w1[:, P * c:P * (c + 1)],
                                         rhs=lnTf[:, c0:c1], start=True, stop=True)
                        u_sb = spool.tile([P, 512], CDT, tag=f"u_sb{c}", bufs=2)
                        nc.scalar.activation(u_sb, up, AF.Relu,
                                             bias=b1p[c][:, 0:1], scale=1.0)
                        u_tiles.append(u_sb)
                    y_ps = fps.tile([P, 512], F32, tag="y_ps", bufs=2)
                    for c in range(4):
                        nc.tensor.matmul(y_ps, lhsT=w2[:, c, :], rhs=u_tiles[c],
                                         start=(c == 0), stop=(c == 3))
                    nc.scalar.activation(yT[:, c0:c1], y_ps, AF.Identity,
                                         bias=b2c[:, 0:1], scale=1.0)
            with tc.tile_pool(name="resf_ps", bufs=2, space="PSUM") as rps:
                _residual_add_T(nc, rps, h_sb, yT, ident_c)

        # ---- output head: y = sigmoid(h @ out_W + out_b) ---- (fp32)
        outw = spool.tile([P, 1], F32, tag="outw")
        nc.sync.dma_start(out=outw, in_=outw_d[:, :])
        outb = spool.tile([1, 1], F32, tag="outb")
        nc.sync.dma_start(out=outb, in_=outb_d[:])
        y_sb = spool.tile([1, S], F32, tag="y_sb")
        with tc.tile_pool(name="out_ps", bufs=2, space="PSUM") as ops:
            for t in range(ST):
                tp = ops.tile([P, P], F32, tag="tp")
                nc.tensor.transpose(tp, h_sb[:, t * P:(t + 1) * P], ident)
                nc.vector.tensor_copy(hT_sb[:, t * P:(t + 1) * P], tp)
            for b in range(NB):
                yp = ops.tile([1, 512], F32, tag="yp")
                nc.tensor.matmul(yp, lhsT=outw, rhs=hT_sb[:, 512 * b:512 * (b + 1)],
                                 start=True, stop=True)
                nc.scalar.activation(y_sb[:, 512 * b:512 * (b + 1)], yp, AF.Sigmoid,
                                     bias=outb[0:1, 0:1], scale=1.0)
        nc.sync.dma_start(out=y_d.rearrange("s o -> o s"), in_=y_sb)

    nc.compile()
    return nc


def kernel(**inputs):
    if "nc" not in _CACHE:
        _CACHE["nc"] = build()
    nc = _CACHE["nc"]

    weight_names = [
        "enc_W", "enc_b", "ln_rnn_a", "ln_rnn_b", "gru_Wih", "gru_Whh",
        "gru_bih", "gru_bhh", "ln_attn_a", "ln_attn_b", "Wq", "bq", "Wk", "bk",
        "Wv", "bv", "Wo", "bo", "ln_ff_a", "ln_ff_b", "ff_W1", "ff_b1",
        "ff_W2", "ff_b2", "out_W", "out_b",
    ]
    shared = {n: np.ascontiguousarray(np.asarray(inputs[n], np.float32))
              for n in weight_names}
    x = np.asarray(inputs["x"], np.float32)
    in_maps = [dict(shared, x=np.ascontiguousarray(x[c])) for c in range(NCORES)]
    res = run_bass_kernel_spmd(nc, in_maps, list(range(NCORES)))
    y = np.stack([res.results[c]["y"] for c in range(NCORES)], axis=0)
    return y.astype(np.float32)
